# revision 3
# baseline (speedup 1.0000x reference)
"""Trainium2 Bass kernel for nn_Attention_16484084483742.

Reference computation (per batch image):
  qkv = x @ Wqkv.T + bqkv            # [N, 3C]
  q, k, v per head (H=12, D=64)
  attn = softmax(q k^T / sqrt(D)) + static_a
  out  = (attn @ v) reassembled -> @ Wproj.T + bproj

Strategy: pure data parallelism over the batch (64 images -> 8 per core,
no collectives). Host pre-transposes x / Wqkv / Wproj / static_a so the
kernel needs no on-chip layout transposes.

Per-core dataflow (b = 8 images, processed as 4 image pairs):
  qkT  [c=1536, tok]   = Wqkv[qk] @ x^T     (float32r matmuls, N=392)
  v    [tok, 768]      = x @ Wqkv[v]^T      (float32r)
  sT   [m, n]          = k_h q_h^T          (bf16, heads paired in PE
                                             row-groups 0-63 / 64-127)
  eT   = exp(sT/8)                          (ACT, straight from PSUM)
  r    = colsum(eT)  via ones-matmul; recip; broadcast to a [128, 392]
         divisor via a tiny indicator matmul
  u    = e^T-weighted v (transposed out)    (bf16; lhsT = v)
  av   = static_a^T-weighted v              (bf16; shares LDWEIGHTS with u)
  ocatT[c, tok] = u * (1/r) + av            (DVE, bf16)
  out  [tok, 768] = ocatT^T @ WprojT + bproj (bf16 matmuls)
"""

import numpy as np
import ml_dtypes

import concourse.bass as bass
import concourse.tile as tile
from concourse import bacc, mybir
from concourse.bass import ds, ts
from concourse.bass_utils import run_bass_kernel_spmd

F32 = mybir.dt.float32
F32R = mybir.dt.float32r
BF16 = mybir.dt.bfloat16

N_CORES = 8
B_PER_CORE = 8
N = 196            # tokens per image
C = 768
H = 12
TOK = B_PER_CORE * N   # 1568 tokens per core
NPAIR = 2 * N          # 392, token columns per image pair
N_PAIRS = B_PER_CORE // 2
KCH = C // 128         # 6 contraction chunks
MQK = 1536 // 128      # 12 output chunks for q,k part

_BUILD_CACHE = {}


def build_nc():
    nc = bacc.Bacc()

    xT_d = nc.dram_tensor("xT", [C, TOK], F32R, kind="ExternalInput")
    wqkvT_d = nc.dram_tensor("wqkvT", [C, 3 * C], F32R, kind="ExternalInput")
    bqkv_d = nc.dram_tensor("bqkv", [3 * C], F32, kind="ExternalInput")
    wprojT_d = nc.dram_tensor("wprojT", [C, C], BF16, kind="ExternalInput")
    bproj_d = nc.dram_tensor("bproj", [C], F32, kind="ExternalInput")
    aT_d = nc.dram_tensor("aT", [H, N, N], BF16, kind="ExternalInput")
    out_d = nc.dram_tensor("out", [TOK, C], F32, kind="ExternalOutput")

    xTr = xT_d.rearrange("(k p) t -> p k t", p=128)
    w1r = wqkvT_d.rearrange("(k p) m -> p k m", p=128)
    wpr = wprojT_d.rearrange("(k p) m -> p k m", p=128)

    with tile.TileContext(nc) as tc:
        with (
            tc.tile_pool(name="const", bufs=1) as const_pool,
            tc.tile_pool(name="xsb", bufs=2) as xpool,
            tc.tile_pool(name="qk", bufs=2) as qkpool,
            tc.tile_pool(name="vp", bufs=2) as vpool,
            tc.tile_pool(name="eT", bufs=4) as epool,
            tc.tile_pool(name="oc", bufs=2) as ocpool,
            tc.tile_pool(name="osb", bufs=3) as opool,
            tc.tile_pool(name="dsb", bufs=2) as dpool,
            tc.tile_pool(name="ps_s", bufs=2, space="PSUM") as ps_s,
            tc.tile_pool(name="ps_rd", bufs=2, space="PSUM") as ps_rd,
            tc.tile_pool(name="ps_uav", bufs=2, space="PSUM") as ps_uav,
            tc.tile_pool(name="ps_mm", bufs=2, space="PSUM") as ps_mm,
        ):
            # ---- resident constants ----
            W1 = const_pool.tile([128, KCH, 3 * C], F32R)
            nc.sync.dma_start(W1[:], w1r[:])
            Wp = const_pool.tile([128, KCH, C], BF16)
            nc.sync.dma_start(Wp[:], wpr[:])
            bqkv_sb = const_pool.tile([128, MQK], F32)
            nc.sync.dma_start(
                bqkv_sb[:], bqkv_d[0:1536].rearrange("(m p) -> p m", p=128)
            )
            aT_sb = const_pool.tile([128, H, 2, N], BF16)
            for h in range(H):
                nc.sync.dma_start(aT_sb[:, h, 0, :], aT_d[h, 0:128, :])
                nc.sync.dma_start(aT_sb[0:68, h, 1, :], aT_d[h, 128:N, :])

            ones_col = const_pool.tile([128, 1], BF16)
            nc.vector.memset(ones_col[:], 1.0)
            ones_row = const_pool.tile([1, 128], F32)
            nc.vector.memset(ones_row[:], 1.0)
            ind = const_pool.tile([33, 128], BF16)
            nc.vector.memset(ind[:], 0.0)
            nc.vector.memset(ind[0:1, 0:64], 1.0)
            nc.vector.memset(ind[32:33, 64:128], 1.0)
            recip_t = const_pool.tile([33, NPAIR], BF16)
            nc.vector.memset(recip_t[:], 0.0)

            # broadcast bias rows to 128 partitions via ones-matmul
            row_v = const_pool.tile([1, C], F32)
            nc.sync.dma_start(
                row_v[:], bqkv_d[1536 : 3 * C].rearrange("(a m) -> a m", a=1)
            )
            row_p = const_pool.tile([1, C], F32)
            nc.sync.dma_start(row_p[:], bproj_d.rearrange("(a m) -> a m", a=1))
            bias_v = const_pool.tile([128, C], F32)
            bias_p = const_pool.tile([128, C], F32)
            for row, dst in ((row_v, bias_v), (row_p, bias_p)):
                for noff, nsz in ((0, 512), (512, 256)):
                    psb = ps_mm.tile([128, 512], F32, tag="mm")
                    nc.tensor.matmul(
                        psb[:, 0:nsz],
                        ones_row[0:1, :],
                        row[0:1, ds(noff, nsz)],
                        start=True,
                        stop=True,
                    )
                    nc.vector.tensor_copy(dst[:, ds(noff, nsz)], psb[:, 0:nsz])

            # ---- main loop over image pairs ----
            for g in range(N_PAIRS):
                gcol = g * NPAIR

                # --- qkv projection (q,k transposed part) ---
                xsb = xpool.tile([128, KCH, NPAIR], F32R)
                for k in range(KCH):
                    nc.sync.dma_start(xsb[:, k, :], xTr[:, k, ds(gcol, NPAIR)])

                qkT = qkpool.tile([128, MQK, NPAIR], BF16)
                for m in range(MQK):
                    ps = ps_mm.tile([128, 512], F32, tag="mm")
                    for k in range(KCH):
                        nc.tensor.matmul(
                            ps[:, 0:NPAIR],
                            W1[:, k, ts(m, 128)],
                            xsb[:, k, :],
                            start=(k == 0),
                            stop=(k == KCH - 1),
                        )
                    nc.scalar.activation(
                        qkT[:, m, :],
                        ps[:, 0:NPAIR],
                        mybir.ActivationFunctionType.Identity,
                        bias=bqkv_sb[:, m : m + 1],
                        scale=1.0,
                    )

                # --- v in natural layout [tok, 768] ---
                v_g = vpool.tile([128, 2, 2, C], BF16)
                for b01 in range(2):
                    for tch, (toff, tm) in enumerate(((0, 128), (128, 68))):
                        ps1 = ps_mm.tile([128, 512], F32, tag="mm")
                        ps2 = ps_mm.tile([128, 512], F32, tag="mm")
                        for k in range(KCH):
                            lhsT = xsb[:, k, ds(b01 * N + toff, tm)]
                            nc.tensor.matmul(
                                ps1[0:tm, 0:512],
                                lhsT,
                                W1[:, k, ds(1536, 512)],
                                start=(k == 0),
                                stop=(k == KCH - 1),
                            )
                            nc.tensor.matmul(
                                ps2[0:tm, 0:256],
                                lhsT,
                                W1[:, k, ds(2048, 256)],
                                start=(k == 0),
                                stop=(k == KCH - 1),
                            )
                        nc.vector.tensor_add(
                            v_g[0:tm, b01, tch, 0:512],
                            ps1[0:tm, 0:512],
                            bias_v[0:tm, 0:512],
                        )
                        nc.vector.tensor_add(
                            v_g[0:tm, b01, tch, 512:768],
                            ps2[0:tm, 0:256],
                            bias_v[0:tm, 512:768],
                        )

                # --- attention, head pairs (2j, 2j+1) ---
                ocat = ocpool.tile([128, KCH, NPAIR], BF16)
                for j in range(KCH):
                    he, ho = 2 * j, 2 * j + 1
                    # scores sT[m, n] per head; even head in partitions 0-63,
                    # odd head in 64-127 (concurrent PE row groups)
                    psA = {}
                    psB = {}
                    for h, base in ((he, 0), (ho, 64)):
                        psA[h] = ps_s.tile([128, NPAIR], F32, tag="sT", name=f"psA{h}")
                        psB[h] = ps_s.tile([128, NPAIR], F32, tag="sT", name=f"psB{h}")
                    for b01 in range(2):
                        bcol = b01 * N
                        for h, base in ((he, 0), (ho, 64)):
                            kk = qkT[ds(base, 64), 6 + j, :]
                            qq = qkT[ds(base, 64), j, ds(bcol, N)]
                            nc.tensor.matmul(
                                psA[h][:, ds(bcol, N)],
                                kk[:, ds(bcol, 128)],
                                qq,
                                start=True,
                                stop=True,
                            )
                            nc.tensor.matmul(
                                psB[h][0:68, ds(bcol, N)],
                                kk[:, ds(bcol + 128, 68)],
                                qq,
                                start=True,
                                stop=True,
                            )
                    eT = {}
                    for h in (he, ho):
                        eT[h] = epool.tile([128, 2, NPAIR], BF16, tag="eT", name=f"eT{h}")
                        nc.scalar.activation(
                            eT[h][:, 0, :],
                            psA[h][:],
                            mybir.ActivationFunctionType.Exp,
                            scale=0.125,
                        )
                        nc.scalar.activation(
                            eT[h][0:68, 1, :],
                            psB[h][0:68, :],
                            mybir.ActivationFunctionType.Exp,
                            scale=0.125,
                        )

                    # r = colsum(eT) via ones matmuls -> rows 0 / 32
                    ps_r = ps_rd.tile([33, NPAIR], F32, tag="rd")
                    for h, row in ((he, 0), (ho, 32)):
                        for b01 in range(2):
                            bcol = b01 * N
                            nc.tensor.matmul(
                                ps_r[row : row + 1, ds(bcol, N)],
                                ones_col[0:128, 0:1],
                                eT[h][:, 0, ds(bcol, N)],
                                start=True,
                                stop=False,
                            )
                            nc.tensor.matmul(
                                ps_r[row : row + 1, ds(bcol, N)],
                                ones_col[0:68, 0:1],
                                eT[h][0:68, 1, ds(bcol, N)],
                                start=False,
                                stop=True,
                            )
                    with nc.allow_low_precision(reason="bf16 softmax recip"):
                        nc.vector.reciprocal(recip_t[0:1, :], ps_r[0:1, :])
                        nc.vector.reciprocal(recip_t[32:33, :], ps_r[32:33, :])

                    ps_div = ps_rd.tile([128, NPAIR], F32, tag="rd")
                    nc.tensor.matmul(
                        ps_div[:], ind[0:33, :], recip_t[0:33, :], start=True, stop=True
                    )
                    div_sb = dpool.tile([128, NPAIR], BF16)
                    nc.scalar.activation(
                        div_sb[:], ps_div[:], mybir.ActivationFunctionType.Copy
                    )

                    # u (e-weighted v, transposed out) and av (static bias term)
                    ps_u = ps_uav.tile([128, NPAIR], F32, tag="uav")
                    ps_av = ps_uav.tile([128, NPAIR], F32, tag="uav")
                    for h, base in ((he, 0), (ho, 64)):
                        for b01 in range(2):
                            bcol = b01 * N
                            for kch, kn in ((0, 128), (1, 68)):
                                vv = v_g[0:kn, b01, kch, ds(h * 64, 64)]
                                nc.tensor.matmul(
                                    ps_u[ds(base, 64), ds(bcol, N)],
                                    vv,
                                    eT[h][0:kn, kch, ds(bcol, N)],
                                    start=(kch == 0),
                                    stop=(kch == 1),
                                )
                                nc.tensor.matmul(
                                    ps_av[ds(base, 64), ds(bcol, N)],
                                    vv,
                                    aT_sb[0:kn, h, kch, :],
                                    start=(kch == 0),
                                    stop=(kch == 1),
                                )
                    nc.vector.tensor_mul(ocat[:, j, :], ps_u[:], div_sb[:])
                    nc.vector.tensor_add(ocat[:, j, :], ocat[:, j, :], ps_av[:])

                # --- output projection [tok, 768] ---
                for m_idx in range(4):
                    b01, half = divmod(m_idx, 2)
                    toff = b01 * N + half * 128
                    tm = 128 if half == 0 else 68
                    pp1 = ps_mm.tile([128, 512], F32, tag="mm")
                    pp2 = ps_mm.tile([128, 512], F32, tag="mm")
                    for j in range(KCH):
                        lhsT = ocat[:, j, ds(toff, tm)]
                        nc.tensor.matmul(
                            pp1[0:tm, 0:512],
                            lhsT,
                            Wp[:, j, 0:512],
                            start=(j == 0),
                            stop=(j == KCH - 1),
                        )
                        nc.tensor.matmul(
                            pp2[0:tm, 0:256],
                            lhsT,
                            Wp[:, j, 512:768],
                            start=(j == 0),
                            stop=(j == KCH - 1),
                        )
                    osb = opool.tile([128, C], F32)
                    nc.vector.tensor_add(
                        osb[0:tm, 0:512], pp1[0:tm, 0:512], bias_p[0:tm, 0:512]
                    )
                    nc.vector.tensor_add(
                        osb[0:tm, 512:768], pp2[0:tm, 0:256], bias_p[0:tm, 512:768]
                    )
                    nc.sync.dma_start(
                        out_d[ds(gcol + toff, tm), :], osb[0:tm, :]
                    )

    nc.compile()
    return nc


def _prep_in_maps(x, Wqkv, bqkv, Wproj, bproj, static_a):
    x = np.asarray(x, dtype=np.float32)
    Wqkv = np.asarray(Wqkv, dtype=np.float32)
    bqkv = np.asarray(bqkv, dtype=np.float32)
    Wproj = np.asarray(Wproj, dtype=np.float32)
    bproj = np.asarray(bproj, dtype=np.float32)
    static_a = np.asarray(static_a, dtype=np.float32)

    wqkvT = np.ascontiguousarray(Wqkv.T)
    wprojT = np.ascontiguousarray(Wproj.T).astype(ml_dtypes.bfloat16)
    aT = np.ascontiguousarray(static_a[0].transpose(0, 2, 1)).astype(
        ml_dtypes.bfloat16
    )

    in_maps = []
    for i in range(N_CORES):
        xc = x[i * B_PER_CORE : (i + 1) * B_PER_CORE]  # [8, 196, 768]
        xT = np.ascontiguousarray(xc.transpose(2, 0, 1).reshape(C, TOK))
        in_maps.append(
            {
                "xT": xT,
                "wqkvT": wqkvT,
                "bqkv": bqkv,
                "wprojT": wprojT,
                "bproj": bproj,
                "aT": aT,
            }
        )
    return in_maps


def kernel(x, Wqkv, bqkv, Wproj, bproj, static_a, _trace=False, _trace_kwargs=None):
    if "nc" not in _BUILD_CACHE:
        _BUILD_CACHE["nc"] = build_nc()
    nc = _BUILD_CACHE["nc"]
    in_maps = _prep_in_maps(x, Wqkv, bqkv, Wproj, bproj, static_a)
    res = run_bass_kernel_spmd(
        nc,
        in_maps,
        core_ids=list(range(N_CORES)),
        trace=_trace,
        **(_trace_kwargs or {}),
    )
    outs = [res.results[i]["out"].reshape(B_PER_CORE, N, C) for i in range(N_CORES)]
    full = np.concatenate(outs, axis=0).astype(np.float32)
    if _trace:
        kernel.last_results = res
    return full


# revision 4
# speedup vs baseline: 1.2668x; 1.2668x over previous
"""Trainium2 Bass kernel for nn_Attention_16484084483742.

Reference computation (per batch image):
  qkv = x @ Wqkv.T + bqkv            # [N, 3C]
  q, k, v per head (H=12, D=64)
  attn = softmax(q k^T / sqrt(D)) + static_a
  out  = (attn @ v) reassembled -> @ Wproj.T + bproj

Strategy: pure data parallelism over the batch (64 images -> 8 per core,
no collectives). Host pre-transposes x / Wqkv / Wproj / static_a so the
kernel needs no on-chip layout transposes.

Per-core dataflow (b = 8 images, processed as 4 image pairs):
  qkT  [c=1536, tok]   = Wqkv[qk] @ x^T     (float32r matmuls, N=392)
  v    [tok, 768]      = x @ Wqkv[v]^T      (float32r)
  sT   [m, n]          = k_h q_h^T          (bf16, heads paired in PE
                                             row-groups 0-63 / 64-127)
  eT   = exp(sT/8)                          (ACT, straight from PSUM)
  r    = colsum(eT)  via ones-matmul; recip; broadcast to a [128, 392]
         divisor via a tiny indicator matmul
  u    = e^T-weighted v (transposed out)    (bf16; lhsT = v)
  av   = static_a^T-weighted v              (bf16; shares LDWEIGHTS with u)
  ocatT[c, tok] = u * (1/r) + av            (DVE, bf16)
  out  [tok, 768] = ocatT^T @ WprojT + bproj (bf16 matmuls)
"""

import numpy as np
import ml_dtypes

import concourse.bass as bass
import concourse.tile as tile
from concourse import bacc, mybir
from concourse.bass import ds, ts
from concourse.bass_utils import run_bass_kernel_spmd

F32 = mybir.dt.float32
F32R = mybir.dt.float32r
BF16 = mybir.dt.bfloat16

N_CORES = 8
B_PER_CORE = 8
N = 196            # tokens per image
C = 768
H = 12
TOK = B_PER_CORE * N   # 1568 tokens per core
NPAIR = 2 * N          # 392, token columns per image pair
N_PAIRS = B_PER_CORE // 2
KCH = C // 128         # 6 contraction chunks
MQK = 1536 // 128      # 12 output chunks for q,k part

_BUILD_CACHE = {}


def build_nc():
    nc = bacc.Bacc()

    xT_d = nc.dram_tensor("xT", [C, TOK], F32R, kind="ExternalInput")
    wqkvT_d = nc.dram_tensor("wqkvT", [C, 3 * C], F32R, kind="ExternalInput")
    bqkv_d = nc.dram_tensor("bqkv", [3 * C], F32, kind="ExternalInput")
    wprojT_d = nc.dram_tensor("wprojT", [C, C], BF16, kind="ExternalInput")
    bproj_d = nc.dram_tensor("bproj", [C], F32, kind="ExternalInput")
    aT_d = nc.dram_tensor("aT", [H, N, N], BF16, kind="ExternalInput")
    out_d = nc.dram_tensor("out", [TOK, C], F32, kind="ExternalOutput")

    xTr = xT_d.rearrange("(k p) t -> p k t", p=128)
    w1r = wqkvT_d.rearrange("(k p) m -> p k m", p=128)
    wpr = wprojT_d.rearrange("(k p) m -> p k m", p=128)

    with tile.TileContext(nc) as tc:
        with (
            tc.tile_pool(name="const", bufs=1) as const_pool,
            tc.tile_pool(name="xsb", bufs=2) as xpool,
            tc.tile_pool(name="qk", bufs=2) as qkpool,
            tc.tile_pool(name="vp", bufs=2) as vpool,
            tc.tile_pool(name="eT", bufs=4) as epool,
            tc.tile_pool(name="oc", bufs=2) as ocpool,
            tc.tile_pool(name="osb", bufs=3) as opool,
            tc.tile_pool(name="dsb", bufs=2) as dpool,
            tc.tile_pool(name="ps_s", bufs=3, space="PSUM") as ps_s,
            tc.tile_pool(name="ps_rd", bufs=1, space="PSUM") as ps_rd,
            tc.tile_pool(name="ps_uav", bufs=2, space="PSUM") as ps_uav,
            tc.tile_pool(name="ps_mm", bufs=2, space="PSUM") as ps_mm,
        ):
            # ---- resident constants ----
            # W1 split per contraction chunk so the first qkv matmuls can
            # start as soon as chunk 0 lands
            W1 = const_pool.tile([128, KCH, 3 * C], F32R)
            for k in range(KCH):
                nc.sync.dma_start(W1[:, k, :], w1r[:, k, :])
            Wp = const_pool.tile([128, KCH, C], BF16)
            nc.sync.dma_start(Wp[:], wpr[:])
            bqkv_sb = const_pool.tile([128, MQK], F32)
            nc.sync.dma_start(
                bqkv_sb[:], bqkv_d[0:1536].rearrange("(m p) -> p m", p=128)
            )
            aT_sb = const_pool.tile([128, H, 2, N], BF16)
            for h in range(H):
                nc.sync.dma_start(aT_sb[:, h, 0, :], aT_d[h, 0:128, :])
                nc.sync.dma_start(aT_sb[0:68, h, 1, :], aT_d[h, 128:N, :])

            ones64 = const_pool.tile([128, 64], BF16)
            nc.vector.memset(ones64[:], 1.0)
            ones_row = const_pool.tile([1, 128], F32)
            nc.vector.memset(ones_row[:], 1.0)

            # broadcast bias rows to 128 partitions via ones-matmul
            row_v = const_pool.tile([1, C], F32)
            nc.sync.dma_start(
                row_v[:], bqkv_d[1536 : 3 * C].rearrange("(a m) -> a m", a=1)
            )
            row_p = const_pool.tile([1, C], F32)
            nc.sync.dma_start(row_p[:], bproj_d.rearrange("(a m) -> a m", a=1))
            bias_v = const_pool.tile([128, C], F32)
            bias_p = const_pool.tile([128, C], F32)
            for row, dst in ((row_v, bias_v), (row_p, bias_p)):
                for noff, nsz in ((0, 512), (512, 256)):
                    psb = ps_mm.tile([128, 512], F32, tag="mm")
                    nc.tensor.matmul(
                        psb[:, 0:nsz],
                        ones_row[0:1, :],
                        row[0:1, ds(noff, nsz)],
                        start=True,
                        stop=True,
                    )
                    nc.vector.tensor_copy(dst[:, ds(noff, nsz)], psb[:, 0:nsz])

            # ---- main loop over image pairs ----
            for g in range(N_PAIRS):
                gcol = g * NPAIR

                # --- qkv projection (q,k transposed part) ---
                xsb = xpool.tile([128, KCH, NPAIR], F32R)
                for k in range(KCH):
                    nc.sync.dma_start(xsb[:, k, :], xTr[:, k, ds(gcol, NPAIR)])

                qkT = qkpool.tile([128, MQK, NPAIR], BF16)
                for m in range(MQK):
                    ps = ps_mm.tile([128, 512], F32, tag="mm")
                    for k in range(KCH):
                        nc.tensor.matmul(
                            ps[:, 0:NPAIR],
                            W1[:, k, ts(m, 128)],
                            xsb[:, k, :],
                            start=(k == 0),
                            stop=(k == KCH - 1),
                        )
                    nc.scalar.activation(
                        qkT[:, m, :],
                        ps[:, 0:NPAIR],
                        mybir.ActivationFunctionType.Identity,
                        bias=bqkv_sb[:, m : m + 1],
                        scale=1.0,
                    )

                # --- v in natural layout [tok, 768] ---
                v_g = vpool.tile([128, 2, 2, C], BF16)
                for b01 in range(2):
                    for tch, (toff, tm) in enumerate(((0, 128), (128, 68))):
                        ps1 = ps_mm.tile([128, 512], F32, tag="mm")
                        ps2 = ps_mm.tile([128, 512], F32, tag="mm")
                        for k in range(KCH):
                            lhsT = xsb[:, k, ds(b01 * N + toff, tm)]
                            nc.tensor.matmul(
                                ps1[0:tm, 0:512],
                                lhsT,
                                W1[:, k, ds(1536, 512)],
                                start=(k == 0),
                                stop=(k == KCH - 1),
                            )
                            nc.tensor.matmul(
                                ps2[0:tm, 0:256],
                                lhsT,
                                W1[:, k, ds(2048, 256)],
                                start=(k == 0),
                                stop=(k == KCH - 1),
                            )
                        nc.vector.tensor_add(
                            v_g[0:tm, b01, tch, 0:512],
                            ps1[0:tm, 0:512],
                            bias_v[0:tm, 0:512],
                        )
                        nc.vector.tensor_add(
                            v_g[0:tm, b01, tch, 512:768],
                            ps2[0:tm, 0:256],
                            bias_v[0:tm, 512:768],
                        )

                # --- attention, head pairs (2j, 2j+1) ---
                ocat = ocpool.tile([128, KCH, NPAIR], BF16)
                for j in range(KCH):
                    he, ho = 2 * j, 2 * j + 1
                    # scores sT[m, n] per head; even head in partitions 0-63,
                    # odd head in 64-127 (concurrent PE row groups)
                    psA = {}
                    psB = {}
                    for h, base in ((he, 0), (ho, 64)):
                        psA[h] = ps_s.tile([128, NPAIR], F32, tag="sT", name=f"psA{h}")
                        psB[h] = ps_s.tile([128, NPAIR], F32, tag="sT", name=f"psB{h}")
                    for b01 in range(2):
                        bcol = b01 * N
                        for h, base in ((he, 0), (ho, 64)):
                            kk = qkT[ds(base, 64), 6 + j, :]
                            qq = qkT[ds(base, 64), j, ds(bcol, N)]
                            nc.tensor.matmul(
                                psA[h][:, ds(bcol, N)],
                                kk[:, ds(bcol, 128)],
                                qq,
                                start=True,
                                stop=True,
                            )
                            nc.tensor.matmul(
                                psB[h][0:68, ds(bcol, N)],
                                kk[:, ds(bcol + 128, 68)],
                                qq,
                                start=True,
                                stop=True,
                            )
                    eT = {}
                    for h in (he, ho):
                        eT[h] = epool.tile([128, 2, NPAIR], BF16, tag="eT", name=f"eT{h}")
                        nc.scalar.activation(
                            eT[h][:, 0, :],
                            psA[h][:],
                            mybir.ActivationFunctionType.Exp,
                            scale=0.125,
                        )
                        nc.scalar.activation(
                            eT[h][0:68, 1, :],
                            psB[h][0:68, :],
                            mybir.ActivationFunctionType.Exp,
                            scale=0.125,
                        )

                    # r = colsum(eT), replicated onto 64 rows per head via
                    # ones64 lhsT; divisor = exp(-ln(r)) on ACT
                    ps_r = ps_rd.tile([128, NPAIR], F32, tag="rd")
                    for h, base in ((he, 0), (ho, 64)):
                        for kch, kn in ((0, 128), (1, 68)):
                            nc.tensor.matmul(
                                ps_r[ds(base, 64), :],
                                ones64[0:kn, :],
                                eT[h][0:kn, kch, :],
                                start=(kch == 0),
                                stop=(kch == 1),
                            )
                    ln_r = dpool.tile([128, NPAIR], F32, tag="lnr")
                    nc.scalar.activation(
                        ln_r[:], ps_r[:], mybir.ActivationFunctionType.Ln
                    )
                    div_sb = dpool.tile([128, NPAIR], BF16, tag="div")
                    nc.scalar.activation(
                        div_sb[:], ln_r[:], mybir.ActivationFunctionType.Exp, scale=-1.0
                    )

                    # u (e-weighted v, transposed out) and av (static bias term)
                    ps_u = ps_uav.tile([128, NPAIR], F32, tag="uav")
                    ps_av = ps_uav.tile([128, NPAIR], F32, tag="uav")
                    for h, base in ((he, 0), (ho, 64)):
                        for b01 in range(2):
                            bcol = b01 * N
                            for kch, kn in ((0, 128), (1, 68)):
                                vv = v_g[0:kn, b01, kch, ds(h * 64, 64)]
                                nc.tensor.matmul(
                                    ps_u[ds(base, 64), ds(bcol, N)],
                                    vv,
                                    eT[h][0:kn, kch, ds(bcol, N)],
                                    start=(kch == 0),
                                    stop=(kch == 1),
                                )
                                nc.tensor.matmul(
                                    ps_av[ds(base, 64), ds(bcol, N)],
                                    vv,
                                    aT_sb[0:kn, h, kch, :],
                                    start=(kch == 0),
                                    stop=(kch == 1),
                                )
                    nc.vector.tensor_mul(ocat[:, j, :], ps_u[:], div_sb[:])
                    nc.vector.tensor_add(ocat[:, j, :], ocat[:, j, :], ps_av[:])

                # --- output projection [tok, 768] ---
                for m_idx in range(4):
                    b01, half = divmod(m_idx, 2)
                    toff = b01 * N + half * 128
                    tm = 128 if half == 0 else 68
                    pp1 = ps_mm.tile([128, 512], F32, tag="mm")
                    pp2 = ps_mm.tile([128, 512], F32, tag="mm")
                    for j in range(KCH):
                        lhsT = ocat[:, j, ds(toff, tm)]
                        nc.tensor.matmul(
                            pp1[0:tm, 0:512],
                            lhsT,
                            Wp[:, j, 0:512],
                            start=(j == 0),
                            stop=(j == KCH - 1),
                        )
                        nc.tensor.matmul(
                            pp2[0:tm, 0:256],
                            lhsT,
                            Wp[:, j, 512:768],
                            start=(j == 0),
                            stop=(j == KCH - 1),
                        )
                    osb = opool.tile([128, C], F32)
                    nc.vector.tensor_add(
                        osb[0:tm, 0:512], pp1[0:tm, 0:512], bias_p[0:tm, 0:512]
                    )
                    nc.vector.tensor_add(
                        osb[0:tm, 512:768], pp2[0:tm, 0:256], bias_p[0:tm, 512:768]
                    )
                    nc.sync.dma_start(
                        out_d[ds(gcol + toff, tm), :], osb[0:tm, :]
                    )

    nc.compile()
    return nc


def _prep_in_maps(x, Wqkv, bqkv, Wproj, bproj, static_a):
    x = np.asarray(x, dtype=np.float32)
    Wqkv = np.asarray(Wqkv, dtype=np.float32)
    bqkv = np.asarray(bqkv, dtype=np.float32)
    Wproj = np.asarray(Wproj, dtype=np.float32)
    bproj = np.asarray(bproj, dtype=np.float32)
    static_a = np.asarray(static_a, dtype=np.float32)

    wqkvT = np.ascontiguousarray(Wqkv.T)
    wprojT = np.ascontiguousarray(Wproj.T).astype(ml_dtypes.bfloat16)
    aT = np.ascontiguousarray(static_a[0].transpose(0, 2, 1)).astype(
        ml_dtypes.bfloat16
    )

    in_maps = []
    for i in range(N_CORES):
        xc = x[i * B_PER_CORE : (i + 1) * B_PER_CORE]  # [8, 196, 768]
        xT = np.ascontiguousarray(xc.transpose(2, 0, 1).reshape(C, TOK))
        in_maps.append(
            {
                "xT": xT,
                "wqkvT": wqkvT,
                "bqkv": bqkv,
                "wprojT": wprojT,
                "bproj": bproj,
                "aT": aT,
            }
        )
    return in_maps


def kernel(x, Wqkv, bqkv, Wproj, bproj, static_a, _trace=False, _trace_kwargs=None):
    if "nc" not in _BUILD_CACHE:
        _BUILD_CACHE["nc"] = build_nc()
    nc = _BUILD_CACHE["nc"]
    in_maps = _prep_in_maps(x, Wqkv, bqkv, Wproj, bproj, static_a)
    res = run_bass_kernel_spmd(
        nc,
        in_maps,
        core_ids=list(range(N_CORES)),
        trace=_trace,
        **(_trace_kwargs or {}),
    )
    outs = [res.results[i]["out"].reshape(B_PER_CORE, N, C) for i in range(N_CORES)]
    full = np.concatenate(outs, axis=0).astype(np.float32)
    if _trace:
        kernel.last_results = res
    return full


# revision 6
# speedup vs baseline: 1.6766x; 1.3235x over previous
"""Trainium2 Bass kernel for nn_Attention_16484084483742.

Reference computation (per batch image):
  qkv = x @ Wqkv.T + bqkv            # [N, 3C]
  q, k, v per head (H=12, D=64)
  attn = softmax(q k^T / sqrt(D)) + static_a
  out  = (attn @ v) reassembled -> @ Wproj.T + bproj

Strategy: pure data parallelism over the batch (64 images -> 8 per core,
no collectives). Host pre-transposes x / Wqkv / Wproj / static_a so the
kernel needs no on-chip layout transposes.

Per-core dataflow (b = 8 images, processed as 4 image pairs):
  qkT  [c=1536, tok]   = Wqkv[qk] @ x^T     (float32r matmuls, N=392)
  v    [tok, 768]      = x @ Wqkv[v]^T      (float32r)
  sT   [m, n]          = k_h q_h^T          (bf16, heads paired in PE
                                             row-groups 0-63 / 64-127)
  eT   = exp(sT/8)                          (ACT, straight from PSUM)
  r    = colsum(eT)  via ones-matmul; recip; broadcast to a [128, 392]
         divisor via a tiny indicator matmul
  u    = e^T-weighted v (transposed out)    (bf16; lhsT = v)
  av   = static_a^T-weighted v              (bf16; shares LDWEIGHTS with u)
  ocatT[c, tok] = u * (1/r) + av            (DVE, bf16)
  out  [tok, 768] = ocatT^T @ WprojT + bproj (bf16 matmuls)
"""

import numpy as np
import ml_dtypes

import concourse.bass as bass
import concourse.tile as tile
from concourse import bacc, mybir
from concourse.bass import ds, ts
from concourse.bass_utils import run_bass_kernel_spmd

F32 = mybir.dt.float32
F32R = mybir.dt.float32r
BF16 = mybir.dt.bfloat16

N_CORES = 8
B_PER_CORE = 8
N = 196            # tokens per image
C = 768
H = 12
TOK = B_PER_CORE * N   # 1568 tokens per core
NPAIR = 2 * N          # 392, token columns per image pair
N_PAIRS = B_PER_CORE // 2
KCH = C // 128         # 6 contraction chunks
MQK = 1536 // 128      # 12 output chunks for q,k part

_BUILD_CACHE = {}


def build_nc():
    nc = bacc.Bacc()

    xT_d = nc.dram_tensor("xT", [C, TOK], F32R, kind="ExternalInput")
    wqkvT_d = nc.dram_tensor("wqkvT", [C, 3 * C], F32R, kind="ExternalInput")
    bqkv_d = nc.dram_tensor("bqkv_qk", [128, MQK], F32, kind="ExternalInput")
    wprojT_d = nc.dram_tensor("wprojT", [C, C], BF16, kind="ExternalInput")
    bias_v_d = nc.dram_tensor("bias_v", [128, C], F32, kind="ExternalInput")
    bias_p_d = nc.dram_tensor("bias_p", [128, C], F32, kind="ExternalInput")
    aT_d = nc.dram_tensor("aTp", [128, H, 2, N], BF16, kind="ExternalInput")
    out_d = nc.dram_tensor("out", [TOK, C], F32, kind="ExternalOutput")

    xTr = xT_d.rearrange("(k p) t -> p k t", p=128)
    w1r = wqkvT_d.rearrange("(k p) m -> p k m", p=128)
    wpr = wprojT_d.rearrange("(k p) m -> p k m", p=128)

    with tile.TileContext(nc) as tc:
        with (
            tc.tile_pool(name="const", bufs=1) as const_pool,
            tc.tile_pool(name="xsb", bufs=2) as xpool,
            tc.tile_pool(name="qk", bufs=2) as qkpool,
            tc.tile_pool(name="vp", bufs=2) as vpool,
            tc.tile_pool(name="eT", bufs=4) as epool,
            tc.tile_pool(name="oc", bufs=2) as ocpool,
            tc.tile_pool(name="osb", bufs=3) as opool,
            tc.tile_pool(name="dsb", bufs=2) as dpool,
            tc.tile_pool(name="ps_s", bufs=3, space="PSUM") as ps_s,
            tc.tile_pool(name="ps_rd", bufs=1, space="PSUM") as ps_rd,
            tc.tile_pool(name="ps_uav", bufs=2, space="PSUM") as ps_uav,
            tc.tile_pool(name="ps_mm", bufs=2, space="PSUM") as ps_mm,
        ):
            # ---- resident constants ----
            # First the tensors gating the first matmuls: x(g=0) and W1,
            # interleaved per contraction chunk; everything else after.
            W1 = const_pool.tile([128, KCH, 3 * C], F32R)
            xsb0 = xpool.tile([128, KCH, NPAIR], F32R, name="xsb")
            for k in range(KCH):
                nc.sync.dma_start(xsb0[:, k, :], xTr[:, k, ds(0, NPAIR)])
                nc.sync.dma_start(W1[:, k, :], w1r[:, k, :])
            Wp = const_pool.tile([128, KCH, C], BF16)
            nc.sync.dma_start(Wp[:], wpr[:])
            bqkv_sb = const_pool.tile([128, MQK], F32)
            nc.sync.dma_start(bqkv_sb[:], bqkv_d[:])
            aT_sb = const_pool.tile([128, H, 2, N], BF16)
            nc.sync.dma_start(aT_sb[:], aT_d[:])
            bias_v = const_pool.tile([128, C], F32)
            nc.sync.dma_start(bias_v[:], bias_v_d[:])
            bias_p = const_pool.tile([128, C], F32)
            nc.sync.dma_start(bias_p[:], bias_p_d[:])

            ones64 = const_pool.tile([128, 64], BF16)
            nc.vector.memset(ones64[:], 1.0)

            # ---- main loop over image pairs ----
            for g in range(N_PAIRS):
                gcol = g * NPAIR

                # --- qkv projection (q,k transposed part) ---
                if g == 0:
                    xsb = xsb0
                else:
                    xsb = xpool.tile([128, KCH, NPAIR], F32R, name="xsb")
                    for k in range(KCH):
                        nc.sync.dma_start(xsb[:, k, :], xTr[:, k, ds(gcol, NPAIR)])

                qkT = qkpool.tile([128, MQK, NPAIR], BF16)
                for m in range(MQK):
                    ps = ps_mm.tile([128, 512], F32, tag="mm")
                    for k in range(KCH):
                        nc.tensor.matmul(
                            ps[:, 0:NPAIR],
                            W1[:, k, ts(m, 128)],
                            xsb[:, k, :],
                            start=(k == 0),
                            stop=(k == KCH - 1),
                        )
                    nc.scalar.activation(
                        qkT[:, m, :],
                        ps[:, 0:NPAIR],
                        mybir.ActivationFunctionType.Identity,
                        bias=bqkv_sb[:, m : m + 1],
                        scale=1.0,
                    )

                # --- v in natural layout [tok, 768] ---
                v_g = vpool.tile([128, 2, 2, C], BF16)
                for b01 in range(2):
                    for tch, (toff, tm) in enumerate(((0, 128), (128, 68))):
                        ps1 = ps_mm.tile([128, 512], F32, tag="mm")
                        ps2 = ps_mm.tile([128, 512], F32, tag="mm")
                        for k in range(KCH):
                            lhsT = xsb[:, k, ds(b01 * N + toff, tm)]
                            nc.tensor.matmul(
                                ps1[0:tm, 0:512],
                                lhsT,
                                W1[:, k, ds(1536, 512)],
                                start=(k == 0),
                                stop=(k == KCH - 1),
                            )
                            nc.tensor.matmul(
                                ps2[0:tm, 0:256],
                                lhsT,
                                W1[:, k, ds(2048, 256)],
                                start=(k == 0),
                                stop=(k == KCH - 1),
                            )
                        nc.vector.tensor_add(
                            v_g[0:tm, b01, tch, 0:512],
                            ps1[0:tm, 0:512],
                            bias_v[0:tm, 0:512],
                        )
                        nc.vector.tensor_add(
                            v_g[0:tm, b01, tch, 512:768],
                            ps2[0:tm, 0:256],
                            bias_v[0:tm, 512:768],
                        )

                # --- attention, head pairs (2j, 2j+1) ---
                ocat = ocpool.tile([128, KCH, NPAIR], BF16)
                for j in range(KCH):
                    he, ho = 2 * j, 2 * j + 1
                    # scores sT[m, n] per head; even head in partitions 0-63,
                    # odd head in 64-127 (concurrent PE row groups)
                    psA = {}
                    psB = {}
                    for h, base in ((he, 0), (ho, 64)):
                        psA[h] = ps_s.tile([128, NPAIR], F32, tag="sT", name=f"psA{h}")
                        psB[h] = ps_s.tile([128, NPAIR], F32, tag="sT", name=f"psB{h}")
                    for b01 in range(2):
                        bcol = b01 * N
                        for h, base in ((he, 0), (ho, 64)):
                            kk = qkT[ds(base, 64), 6 + j, :]
                            qq = qkT[ds(base, 64), j, ds(bcol, N)]
                            nc.tensor.matmul(
                                psA[h][:, ds(bcol, N)],
                                kk[:, ds(bcol, 128)],
                                qq,
                                start=True,
                                stop=True,
                            )
                            nc.tensor.matmul(
                                psB[h][0:68, ds(bcol, N)],
                                kk[:, ds(bcol + 128, 68)],
                                qq,
                                start=True,
                                stop=True,
                            )
                    eT = {}
                    for h in (he, ho):
                        eT[h] = epool.tile([128, 2, NPAIR], BF16, tag="eT", name=f"eT{h}")
                        nc.scalar.activation(
                            eT[h][:, 0, :],
                            psA[h][:],
                            mybir.ActivationFunctionType.Exp,
                            scale=0.125,
                        )
                        nc.scalar.activation(
                            eT[h][0:68, 1, :],
                            psB[h][0:68, :],
                            mybir.ActivationFunctionType.Exp,
                            scale=0.125,
                        )

                    # r = colsum(eT), replicated onto 64 rows per head via
                    # ones64 lhsT; divisor = exp(-ln(r)) on ACT
                    ps_r = ps_rd.tile([128, NPAIR], F32, tag="rd")
                    for h, base in ((he, 0), (ho, 64)):
                        for kch, kn in ((0, 128), (1, 68)):
                            nc.tensor.matmul(
                                ps_r[ds(base, 64), :],
                                ones64[0:kn, :],
                                eT[h][0:kn, kch, :],
                                start=(kch == 0),
                                stop=(kch == 1),
                            )
                    div_sb = dpool.tile([128, NPAIR], F32, tag="div")
                    nc.vector.reciprocal_approx_fast(div_sb[:], ps_r[:])

                    # u (e-weighted v, transposed out) and av (static bias term)
                    ps_u = ps_uav.tile([128, NPAIR], F32, tag="uav")
                    ps_av = ps_uav.tile([128, NPAIR], F32, tag="uav")
                    for h, base in ((he, 0), (ho, 64)):
                        for b01 in range(2):
                            bcol = b01 * N
                            for kch, kn in ((0, 128), (1, 68)):
                                vv = v_g[0:kn, b01, kch, ds(h * 64, 64)]
                                nc.tensor.matmul(
                                    ps_u[ds(base, 64), ds(bcol, N)],
                                    vv,
                                    eT[h][0:kn, kch, ds(bcol, N)],
                                    start=(kch == 0),
                                    stop=(kch == 1),
                                )
                                nc.tensor.matmul(
                                    ps_av[ds(base, 64), ds(bcol, N)],
                                    vv,
                                    aT_sb[0:kn, h, kch, :],
                                    start=(kch == 0),
                                    stop=(kch == 1),
                                )
                    nc.vector.tensor_mul(ocat[:, j, :], ps_u[:], div_sb[:])
                    nc.vector.tensor_add(ocat[:, j, :], ocat[:, j, :], ps_av[:])

                # --- output projection [tok, 768] ---
                for m_idx in range(4):
                    b01, half = divmod(m_idx, 2)
                    toff = b01 * N + half * 128
                    tm = 128 if half == 0 else 68
                    pp1 = ps_mm.tile([128, 512], F32, tag="mm")
                    pp2 = ps_mm.tile([128, 512], F32, tag="mm")
                    for j in range(KCH):
                        lhsT = ocat[:, j, ds(toff, tm)]
                        nc.tensor.matmul(
                            pp1[0:tm, 0:512],
                            lhsT,
                            Wp[:, j, 0:512],
                            start=(j == 0),
                            stop=(j == KCH - 1),
                        )
                        nc.tensor.matmul(
                            pp2[0:tm, 0:256],
                            lhsT,
                            Wp[:, j, 512:768],
                            start=(j == 0),
                            stop=(j == KCH - 1),
                        )
                    osb = opool.tile([128, C], F32)
                    nc.vector.tensor_add(
                        osb[0:tm, 0:512], pp1[0:tm, 0:512], bias_p[0:tm, 0:512]
                    )
                    nc.vector.tensor_add(
                        osb[0:tm, 512:768], pp2[0:tm, 0:256], bias_p[0:tm, 512:768]
                    )
                    nc.sync.dma_start(
                        out_d[ds(gcol + toff, tm), :], osb[0:tm, :]
                    )

    nc.compile()
    return nc


def _prep_in_maps(x, Wqkv, bqkv, Wproj, bproj, static_a):
    x = np.asarray(x, dtype=np.float32)
    Wqkv = np.asarray(Wqkv, dtype=np.float32)
    bqkv = np.asarray(bqkv, dtype=np.float32)
    Wproj = np.asarray(Wproj, dtype=np.float32)
    bproj = np.asarray(bproj, dtype=np.float32)
    static_a = np.asarray(static_a, dtype=np.float32)

    wqkvT = np.ascontiguousarray(Wqkv.T)
    wprojT = np.ascontiguousarray(Wproj.T).astype(ml_dtypes.bfloat16)
    # aT packed for single-DMA load: aTp[p, h, ch, n] = static_a[0,h].T[ch*128+p, n]
    aTt = static_a[0].transpose(0, 2, 1)  # [H, m, n]
    aTp = np.zeros((128, H, 2, N), dtype=np.float32)
    aTp[:, :, 0, :] = aTt.transpose(1, 0, 2)[0:128]
    aTp[0:68, :, 1, :] = aTt.transpose(1, 0, 2)[128:N]
    aTp = aTp.astype(ml_dtypes.bfloat16)
    bqkv_qk = np.ascontiguousarray(bqkv[0:1536].reshape(12, 128).T)
    bias_v = np.broadcast_to(bqkv[1536:], (128, C)).copy()
    bias_p = np.broadcast_to(bproj, (128, C)).copy()

    in_maps = []
    for i in range(N_CORES):
        xc = x[i * B_PER_CORE : (i + 1) * B_PER_CORE]  # [8, 196, 768]
        xT = np.ascontiguousarray(xc.transpose(2, 0, 1).reshape(C, TOK))
        in_maps.append(
            {
                "xT": xT,
                "wqkvT": wqkvT,
                "bqkv_qk": bqkv_qk,
                "wprojT": wprojT,
                "bias_v": bias_v,
                "bias_p": bias_p,
                "aTp": aTp,
            }
        )
    return in_maps


def kernel(x, Wqkv, bqkv, Wproj, bproj, static_a, _trace=False, _trace_kwargs=None):
    if "nc" not in _BUILD_CACHE:
        _BUILD_CACHE["nc"] = build_nc()
    nc = _BUILD_CACHE["nc"]
    in_maps = _prep_in_maps(x, Wqkv, bqkv, Wproj, bproj, static_a)
    res = run_bass_kernel_spmd(
        nc,
        in_maps,
        core_ids=list(range(N_CORES)),
        trace=_trace,
        **(_trace_kwargs or {}),
    )
    outs = [res.results[i]["out"].reshape(B_PER_CORE, N, C) for i in range(N_CORES)]
    full = np.concatenate(outs, axis=0).astype(np.float32)
    if _trace:
        kernel.last_results = res
    return full


# revision 7
# speedup vs baseline: 1.7252x; 1.0289x over previous
"""Trainium2 Bass kernel for nn_Attention_16484084483742.

Reference computation (per batch image):
  qkv = x @ Wqkv.T + bqkv            # [N, 3C]
  q, k, v per head (H=12, D=64)
  attn = softmax(q k^T / sqrt(D)) + static_a
  out  = (attn @ v) reassembled -> @ Wproj.T + bproj

Strategy: pure data parallelism over the batch (64 images -> 8 per core,
no collectives). Host pre-transposes x / Wqkv / Wproj / static_a so the
kernel needs no on-chip layout transposes.

Per-core dataflow (b = 8 images, processed as 4 image pairs):
  qkT  [c=1536, tok]   = Wqkv[qk] @ x^T     (float32r matmuls, N=392)
  v    [tok, 768]      = x @ Wqkv[v]^T      (float32r)
  sT   [m, n]          = k_h q_h^T          (bf16, heads paired in PE
                                             row-groups 0-63 / 64-127)
  eT   = exp(sT/8)                          (ACT, straight from PSUM)
  r    = colsum(eT)  via ones-matmul; recip; broadcast to a [128, 392]
         divisor via a tiny indicator matmul
  u    = e^T-weighted v (transposed out)    (bf16; lhsT = v)
  av   = static_a^T-weighted v              (bf16; shares LDWEIGHTS with u)
  ocatT[c, tok] = u * (1/r) + av            (DVE, bf16)
  out  [tok, 768] = ocatT^T @ WprojT + bproj (bf16 matmuls)
"""

import numpy as np
import ml_dtypes

import concourse.bass as bass
import concourse.tile as tile
from concourse import bacc, mybir
from concourse.bass import ds, ts
from concourse.bass_utils import run_bass_kernel_spmd

F32 = mybir.dt.float32
F32R = mybir.dt.float32r
BF16 = mybir.dt.bfloat16

N_CORES = 8
B_PER_CORE = 8
N = 196            # tokens per image
C = 768
H = 12
TOK = B_PER_CORE * N   # 1568 tokens per core
NPAIR = 2 * N          # 392, token columns per image pair
N_PAIRS = B_PER_CORE // 2
KCH = C // 128         # 6 contraction chunks
MQK = 1536 // 128      # 12 output chunks for q,k part

_BUILD_CACHE = {}


def build_nc():
    nc = bacc.Bacc()

    xT_d = nc.dram_tensor("xT", [C, TOK], F32R, kind="ExternalInput")
    wqkvT_d = nc.dram_tensor("wqkvT", [C, 3 * C], F32R, kind="ExternalInput")
    bqkv_d = nc.dram_tensor("bqkv_qk", [128, MQK], F32, kind="ExternalInput")
    wprojT_d = nc.dram_tensor("wprojT", [C, C], BF16, kind="ExternalInput")
    bias_v_d = nc.dram_tensor("bias_v", [128, C], F32, kind="ExternalInput")
    bias_p_d = nc.dram_tensor("bias_p", [128, C], F32, kind="ExternalInput")
    aT_d = nc.dram_tensor("aTp", [128, H, 2, N], BF16, kind="ExternalInput")
    out_d = nc.dram_tensor("out", [TOK, C], F32, kind="ExternalOutput")

    xTr = xT_d.rearrange("(k p) t -> p k t", p=128)
    w1r = wqkvT_d.rearrange("(k p) m -> p k m", p=128)
    wpr = wprojT_d.rearrange("(k p) m -> p k m", p=128)

    with tile.TileContext(nc) as tc:
        with (
            tc.tile_pool(name="const", bufs=1) as const_pool,
            tc.tile_pool(name="xsb", bufs=2) as xpool,
            tc.tile_pool(name="qk", bufs=2) as qkpool,
            tc.tile_pool(name="vp", bufs=2) as vpool,
            tc.tile_pool(name="eT", bufs=4) as epool,
            tc.tile_pool(name="oc", bufs=2) as ocpool,
            tc.tile_pool(name="osb", bufs=3) as opool,
            tc.tile_pool(name="dsb", bufs=2) as dpool,
            tc.tile_pool(name="ps_s", bufs=3, space="PSUM") as ps_s,
            tc.tile_pool(name="ps_uav", bufs=2, space="PSUM") as ps_uav,
            tc.tile_pool(name="ps_mm", bufs=3, space="PSUM") as ps_mm,
        ):
            # ---- resident constants ----
            # First the tensors gating the first matmuls: x(g=0) and W1,
            # interleaved per contraction chunk; everything else after.
            W1 = const_pool.tile([128, KCH, 3 * C], F32R)
            xsb0 = xpool.tile([128, KCH, NPAIR], F32R, name="xsb")
            for k in range(KCH):
                nc.sync.dma_start(xsb0[:, k, :], xTr[:, k, ds(0, NPAIR)])
                nc.sync.dma_start(W1[:, k, :], w1r[:, k, :])
            Wp = const_pool.tile([128, KCH, C], BF16)
            nc.sync.dma_start(Wp[:], wpr[:])
            bqkv_sb = const_pool.tile([128, MQK], F32)
            nc.sync.dma_start(bqkv_sb[:], bqkv_d[:])
            aT_sb = const_pool.tile([128, H, 2, N], BF16)
            nc.sync.dma_start(aT_sb[:], aT_d[:])
            bias_v = const_pool.tile([128, C], F32)
            nc.sync.dma_start(bias_v[:], bias_v_d[:])
            bias_p = const_pool.tile([128, C], F32)
            nc.sync.dma_start(bias_p[:], bias_p_d[:])

            ones64 = const_pool.tile([128, 64], BF16)
            nc.vector.memset(ones64[:], 1.0)

            # ---- main loop over image pairs ----
            for g in range(N_PAIRS):
                gcol = g * NPAIR

                # --- qkv projection (q,k transposed part) ---
                if g == 0:
                    xsb = xsb0
                else:
                    xsb = xpool.tile([128, KCH, NPAIR], F32R, name="xsb")
                    for k in range(KCH):
                        nc.sync.dma_start(xsb[:, k, :], xTr[:, k, ds(gcol, NPAIR)])

                # --- v in natural layout [tok, 768] ---
                v_g = vpool.tile([128, 2, 2, C], BF16)
                for b01 in range(2):
                    for tch, (toff, tm) in enumerate(((0, 128), (128, 68))):
                        ps1 = ps_mm.tile([128, 512], F32, tag="mm")
                        ps2 = ps_mm.tile([128, 512], F32, tag="mm")
                        for k in range(KCH):
                            lhsT = xsb[:, k, ds(b01 * N + toff, tm)]
                            nc.tensor.matmul(
                                ps1[0:tm, 0:512],
                                lhsT,
                                W1[:, k, ds(1536, 512)],
                                start=(k == 0),
                                stop=(k == KCH - 1),
                            )
                            nc.tensor.matmul(
                                ps2[0:tm, 0:256],
                                lhsT,
                                W1[:, k, ds(2048, 256)],
                                start=(k == 0),
                                stop=(k == KCH - 1),
                            )
                        nc.vector.tensor_add(
                            v_g[0:tm, b01, tch, 0:512],
                            ps1[0:tm, 0:512],
                            bias_v[0:tm, 0:512],
                        )
                        nc.vector.tensor_add(
                            v_g[0:tm, b01, tch, 512:768],
                            ps2[0:tm, 0:256],
                            bias_v[0:tm, 512:768],
                        )

                qkT = qkpool.tile([128, MQK, NPAIR], BF16)
                for m in [0, 6, 1, 7, 2, 8, 3, 9, 4, 10, 5, 11]:
                    ps = ps_mm.tile([128, 512], F32, tag="mm")
                    for k in range(KCH):
                        nc.tensor.matmul(
                            ps[:, 0:NPAIR],
                            W1[:, k, ts(m, 128)],
                            xsb[:, k, :],
                            start=(k == 0),
                            stop=(k == KCH - 1),
                        )
                    nc.scalar.activation(
                        qkT[:, m, :],
                        ps[:, 0:NPAIR],
                        mybir.ActivationFunctionType.Identity,
                        bias=bqkv_sb[:, m : m + 1],
                        scale=1.0,
                    )

                # --- attention, head pairs (2j, 2j+1) ---
                ocat = ocpool.tile([128, KCH, NPAIR], BF16)
                for j in range(KCH):
                    he, ho = 2 * j, 2 * j + 1
                    # scores sT[m, n] per head; even head in partitions 0-63,
                    # odd head in 64-127 (concurrent PE row groups)
                    psA = {}
                    psB = {}
                    for h, base in ((he, 0), (ho, 64)):
                        psA[h] = ps_s.tile([128, NPAIR], F32, tag="sT", name=f"psA{h}")
                        psB[h] = ps_s.tile([128, NPAIR], F32, tag="sT", name=f"psB{h}")
                    for b01 in range(2):
                        bcol = b01 * N
                        for h, base in ((he, 0), (ho, 64)):
                            kk = qkT[ds(base, 64), 6 + j, :]
                            qq = qkT[ds(base, 64), j, ds(bcol, N)]
                            nc.tensor.matmul(
                                psA[h][:, ds(bcol, N)],
                                kk[:, ds(bcol, 128)],
                                qq,
                                start=True,
                                stop=True,
                            )
                            nc.tensor.matmul(
                                psB[h][0:68, ds(bcol, N)],
                                kk[:, ds(bcol + 128, 68)],
                                qq,
                                start=True,
                                stop=True,
                            )
                    eT = {}
                    for h in (he, ho):
                        eT[h] = epool.tile([128, 2, NPAIR], BF16, tag="eT", name=f"eT{h}")
                        nc.scalar.activation(
                            eT[h][:, 0, :],
                            psA[h][:],
                            mybir.ActivationFunctionType.Exp,
                            scale=0.125,
                        )
                        nc.scalar.activation(
                            eT[h][0:68, 1, :],
                            psB[h][0:68, :],
                            mybir.ActivationFunctionType.Exp,
                            scale=0.125,
                        )

                    # r = colsum(eT), replicated onto 64 rows per head via
                    # ones64 lhsT; divisor = exp(-ln(r)) on ACT
                    ps_r = ps_s.tile([128, NPAIR], F32, tag="sT", name="ps_r")
                    for h, base in ((he, 0), (ho, 64)):
                        for kch, kn in ((0, 128), (1, 68)):
                            nc.tensor.matmul(
                                ps_r[ds(base, 64), :],
                                ones64[0:kn, :],
                                eT[h][0:kn, kch, :],
                                start=(kch == 0),
                                stop=(kch == 1),
                            )
                    div_sb = dpool.tile([128, NPAIR], F32, tag="div")
                    nc.vector.reciprocal_approx_fast(div_sb[:], ps_r[:])

                    # u (e-weighted v, transposed out) and av (static bias term)
                    ps_u = ps_uav.tile([128, NPAIR], F32, tag="uav")
                    ps_av = ps_uav.tile([128, NPAIR], F32, tag="uav")
                    for h, base in ((he, 0), (ho, 64)):
                        for b01 in range(2):
                            bcol = b01 * N
                            for kch, kn in ((0, 128), (1, 68)):
                                vv = v_g[0:kn, b01, kch, ds(h * 64, 64)]
                                nc.tensor.matmul(
                                    ps_u[ds(base, 64), ds(bcol, N)],
                                    vv,
                                    eT[h][0:kn, kch, ds(bcol, N)],
                                    start=(kch == 0),
                                    stop=(kch == 1),
                                )
                                nc.tensor.matmul(
                                    ps_av[ds(base, 64), ds(bcol, N)],
                                    vv,
                                    aT_sb[0:kn, h, kch, :],
                                    start=(kch == 0),
                                    stop=(kch == 1),
                                )
                    nc.vector.tensor_mul(ocat[:, j, :], ps_u[:], div_sb[:])
                    nc.vector.tensor_add(ocat[:, j, :], ocat[:, j, :], ps_av[:])

                # --- output projection [tok, 768] ---
                for m_idx in range(4):
                    b01, half = divmod(m_idx, 2)
                    toff = b01 * N + half * 128
                    tm = 128 if half == 0 else 68
                    pp1 = ps_mm.tile([128, 512], F32, tag="mm")
                    pp2 = ps_mm.tile([128, 512], F32, tag="mm")
                    for j in range(KCH):
                        lhsT = ocat[:, j, ds(toff, tm)]
                        nc.tensor.matmul(
                            pp1[0:tm, 0:512],
                            lhsT,
                            Wp[:, j, 0:512],
                            start=(j == 0),
                            stop=(j == KCH - 1),
                        )
                        nc.tensor.matmul(
                            pp2[0:tm, 0:256],
                            lhsT,
                            Wp[:, j, 512:768],
                            start=(j == 0),
                            stop=(j == KCH - 1),
                        )
                    osb = opool.tile([128, C], F32)
                    nc.vector.tensor_add(
                        osb[0:tm, 0:512], pp1[0:tm, 0:512], bias_p[0:tm, 0:512]
                    )
                    nc.vector.tensor_add(
                        osb[0:tm, 512:768], pp2[0:tm, 0:256], bias_p[0:tm, 512:768]
                    )
                    nc.sync.dma_start(
                        out_d[ds(gcol + toff, tm), :], osb[0:tm, :]
                    )

    nc.compile()
    return nc


def _prep_in_maps(x, Wqkv, bqkv, Wproj, bproj, static_a):
    x = np.asarray(x, dtype=np.float32)
    Wqkv = np.asarray(Wqkv, dtype=np.float32)
    bqkv = np.asarray(bqkv, dtype=np.float32)
    Wproj = np.asarray(Wproj, dtype=np.float32)
    bproj = np.asarray(bproj, dtype=np.float32)
    static_a = np.asarray(static_a, dtype=np.float32)

    wqkvT = np.ascontiguousarray(Wqkv.T)
    wprojT = np.ascontiguousarray(Wproj.T).astype(ml_dtypes.bfloat16)
    # aT packed for single-DMA load: aTp[p, h, ch, n] = static_a[0,h].T[ch*128+p, n]
    aTt = static_a[0].transpose(0, 2, 1)  # [H, m, n]
    aTp = np.zeros((128, H, 2, N), dtype=np.float32)
    aTp[:, :, 0, :] = aTt.transpose(1, 0, 2)[0:128]
    aTp[0:68, :, 1, :] = aTt.transpose(1, 0, 2)[128:N]
    aTp = aTp.astype(ml_dtypes.bfloat16)
    bqkv_qk = np.ascontiguousarray(bqkv[0:1536].reshape(12, 128).T)
    bias_v = np.broadcast_to(bqkv[1536:], (128, C)).copy()
    bias_p = np.broadcast_to(bproj, (128, C)).copy()

    in_maps = []
    for i in range(N_CORES):
        xc = x[i * B_PER_CORE : (i + 1) * B_PER_CORE]  # [8, 196, 768]
        xT = np.ascontiguousarray(xc.transpose(2, 0, 1).reshape(C, TOK))
        in_maps.append(
            {
                "xT": xT,
                "wqkvT": wqkvT,
                "bqkv_qk": bqkv_qk,
                "wprojT": wprojT,
                "bias_v": bias_v,
                "bias_p": bias_p,
                "aTp": aTp,
            }
        )
    return in_maps


def kernel(x, Wqkv, bqkv, Wproj, bproj, static_a, _trace=False, _trace_kwargs=None):
    if "nc" not in _BUILD_CACHE:
        _BUILD_CACHE["nc"] = build_nc()
    nc = _BUILD_CACHE["nc"]
    in_maps = _prep_in_maps(x, Wqkv, bqkv, Wproj, bproj, static_a)
    res = run_bass_kernel_spmd(
        nc,
        in_maps,
        core_ids=list(range(N_CORES)),
        trace=_trace,
        **(_trace_kwargs or {}),
    )
    outs = [res.results[i]["out"].reshape(B_PER_CORE, N, C) for i in range(N_CORES)]
    full = np.concatenate(outs, axis=0).astype(np.float32)
    if _trace:
        kernel.last_results = res
    return full


# revision 8
# speedup vs baseline: 1.8412x; 1.0673x over previous
"""Trainium2 Bass kernel for nn_Attention_16484084483742.

Reference computation (per batch image):
  qkv = x @ Wqkv.T + bqkv            # [N, 3C]
  q, k, v per head (H=12, D=64)
  attn = softmax(q k^T / sqrt(D)) + static_a
  out  = (attn @ v) reassembled -> @ Wproj.T + bproj

Strategy: pure data parallelism over the batch (64 images -> 8 per core,
no collectives). Host pre-transposes x / Wqkv / Wproj / static_a so the
kernel needs no on-chip layout transposes.

Per-core dataflow (b = 8 images, processed as 4 image pairs):
  qkT  [c=1536, tok]   = Wqkv[qk] @ x^T     (float32r matmuls, N=392)
  v    [tok, 768]      = x @ Wqkv[v]^T      (float32r)
  sT   [m, n]          = k_h q_h^T          (bf16, heads paired in PE
                                             row-groups 0-63 / 64-127)
  eT   = exp(sT/8)                          (ACT, straight from PSUM)
  r    = colsum(eT)  via ones-matmul; recip; broadcast to a [128, 392]
         divisor via a tiny indicator matmul
  u    = e^T-weighted v (transposed out)    (bf16; lhsT = v)
  av   = static_a^T-weighted v              (bf16; shares LDWEIGHTS with u)
  ocatT[c, tok] = u * (1/r) + av            (DVE, bf16)
  out  [tok, 768] = ocatT^T @ WprojT + bproj (bf16 matmuls)
"""

import numpy as np
import ml_dtypes

import concourse.bass as bass
import concourse.tile as tile
from concourse import bacc, mybir
from concourse.bass import ds, ts
from concourse.bass_utils import run_bass_kernel_spmd

F32 = mybir.dt.float32
F32R = mybir.dt.float32r
BF16 = mybir.dt.bfloat16

N_CORES = 8
B_PER_CORE = 8
N = 196            # tokens per image
C = 768
H = 12
TOK = B_PER_CORE * N   # 1568 tokens per core
NPAIR = 2 * N          # 392, token columns per image pair
N_PAIRS = B_PER_CORE // 2
KCH = C // 128         # 6 contraction chunks
MQK = 1536 // 128      # 12 output chunks for q,k part

_BUILD_CACHE = {}


def build_nc():
    nc = bacc.Bacc()

    xT_d = nc.dram_tensor("xT", [C, TOK], BF16, kind="ExternalInput")
    wqkvT_d = nc.dram_tensor("wqkvT", [C, 3 * C], BF16, kind="ExternalInput")
    bqkv_d = nc.dram_tensor("bqkv_qk", [128, MQK], F32, kind="ExternalInput")
    wprojT_d = nc.dram_tensor("wprojT", [C, C], BF16, kind="ExternalInput")
    bias_v_d = nc.dram_tensor("bias_v", [128, C], F32, kind="ExternalInput")
    bias_p_d = nc.dram_tensor("bias_p", [128, C], F32, kind="ExternalInput")
    aT_d = nc.dram_tensor("aTp", [128, H, 2, N], BF16, kind="ExternalInput")
    out_d = nc.dram_tensor("out", [TOK, C], F32, kind="ExternalOutput")

    xTr = xT_d.rearrange("(k p) t -> p k t", p=128)
    w1r = wqkvT_d.rearrange("(k p) m -> p k m", p=128)
    wpr = wprojT_d.rearrange("(k p) m -> p k m", p=128)

    with tile.TileContext(nc) as tc:
        with (
            tc.tile_pool(name="const", bufs=1) as const_pool,
            tc.tile_pool(name="xsb", bufs=2) as xpool,
            tc.tile_pool(name="qk", bufs=2) as qkpool,
            tc.tile_pool(name="vp", bufs=2) as vpool,
            tc.tile_pool(name="eT", bufs=6) as epool,
            tc.tile_pool(name="oc", bufs=2) as ocpool,
            tc.tile_pool(name="osb", bufs=3) as opool,
            tc.tile_pool(name="dsb", bufs=2) as dpool,
            tc.tile_pool(name="ps_s", bufs=3, space="PSUM") as ps_s,
            tc.tile_pool(name="ps_uav", bufs=2, space="PSUM") as ps_uav,
            tc.tile_pool(name="ps_mm", bufs=3, space="PSUM") as ps_mm,
        ):
            # ---- resident constants ----
            # First the tensors gating the first matmuls: x(g=0) and W1,
            # interleaved per contraction chunk; everything else after.
            W1 = const_pool.tile([128, KCH, 3 * C], BF16)
            xsb0 = xpool.tile([128, KCH, NPAIR], BF16, name="xsb")
            for k in range(KCH):
                nc.sync.dma_start(xsb0[:, k, :], xTr[:, k, ds(0, NPAIR)])
                nc.sync.dma_start(W1[:, k, :], w1r[:, k, :])
            Wp = const_pool.tile([128, KCH, C], BF16)
            nc.sync.dma_start(Wp[:], wpr[:])
            bqkv_sb = const_pool.tile([128, MQK], F32)
            nc.sync.dma_start(bqkv_sb[:], bqkv_d[:])
            aT_sb = const_pool.tile([128, H, 2, N], BF16)
            nc.sync.dma_start(aT_sb[:], aT_d[:])
            bias_v = const_pool.tile([128, C], F32)
            nc.sync.dma_start(bias_v[:], bias_v_d[:])
            bias_p = const_pool.tile([128, C], F32)
            nc.sync.dma_start(bias_p[:], bias_p_d[:])

            ones64 = const_pool.tile([128, 64], BF16)
            nc.vector.memset(ones64[:], 1.0)

            # ---- main loop over image pairs ----
            for g in range(N_PAIRS):
                gcol = g * NPAIR

                # --- qkv projection (q,k transposed part) ---
                if g == 0:
                    xsb = xsb0
                else:
                    xsb = xpool.tile([128, KCH, NPAIR], BF16, name="xsb")
                    for k in range(KCH):
                        nc.sync.dma_start(xsb[:, k, :], xTr[:, k, ds(gcol, NPAIR)])

                # --- v in natural layout [tok, 768] ---
                v_g = vpool.tile([128, 2, 2, C], BF16)
                for b01 in range(2):
                    for tch, (toff, tm) in enumerate(((0, 128), (128, 68))):
                        ps1 = ps_mm.tile([128, 512], F32, tag="mm")
                        ps2 = ps_mm.tile([128, 512], F32, tag="mm")
                        for k in range(KCH):
                            lhsT = xsb[:, k, ds(b01 * N + toff, tm)]
                            nc.tensor.matmul(
                                ps1[0:tm, 0:512],
                                lhsT,
                                W1[:, k, ds(1536, 512)],
                                start=(k == 0),
                                stop=(k == KCH - 1),
                            )
                            nc.tensor.matmul(
                                ps2[0:tm, 0:256],
                                lhsT,
                                W1[:, k, ds(2048, 256)],
                                start=(k == 0),
                                stop=(k == KCH - 1),
                            )
                        nc.vector.tensor_add(
                            v_g[0:tm, b01, tch, 0:512],
                            ps1[0:tm, 0:512],
                            bias_v[0:tm, 0:512],
                        )
                        nc.vector.tensor_add(
                            v_g[0:tm, b01, tch, 512:768],
                            ps2[0:tm, 0:256],
                            bias_v[0:tm, 512:768],
                        )

                qkT = qkpool.tile([128, MQK, NPAIR], BF16)
                for m in [0, 6, 1, 7, 2, 8, 3, 9, 4, 10, 5, 11]:
                    ps = ps_mm.tile([128, 512], F32, tag="mm")
                    for k in range(KCH):
                        nc.tensor.matmul(
                            ps[:, 0:NPAIR],
                            W1[:, k, ts(m, 128)],
                            xsb[:, k, :],
                            start=(k == 0),
                            stop=(k == KCH - 1),
                        )
                    nc.scalar.activation(
                        qkT[:, m, :],
                        ps[:, 0:NPAIR],
                        mybir.ActivationFunctionType.Identity,
                        bias=bqkv_sb[:, m : m + 1],
                        scale=1.0,
                    )

                # --- attention, head pairs (2j, 2j+1) ---
                ocat = ocpool.tile([128, KCH, NPAIR], BF16)
                for j in range(KCH):
                    he, ho = 2 * j, 2 * j + 1
                    # scores sT[m, n] per head; even head in partitions 0-63,
                    # odd head in 64-127 (concurrent PE row groups)
                    psA = {}
                    psB = {}
                    for h, base in ((he, 0), (ho, 64)):
                        psA[h] = ps_s.tile([128, NPAIR], F32, tag="sT", name=f"psA{h}")
                        psB[h] = ps_s.tile([128, NPAIR], F32, tag="sT", name=f"psB{h}")
                    for b01 in range(2):
                        bcol = b01 * N
                        for h, base in ((he, 0), (ho, 64)):
                            kk = qkT[ds(base, 64), 6 + j, :]
                            qq = qkT[ds(base, 64), j, ds(bcol, N)]
                            nc.tensor.matmul(
                                psA[h][:, ds(bcol, N)],
                                kk[:, ds(bcol, 128)],
                                qq,
                                start=True,
                                stop=True,
                            )
                            nc.tensor.matmul(
                                psB[h][0:68, ds(bcol, N)],
                                kk[:, ds(bcol + 128, 68)],
                                qq,
                                start=True,
                                stop=True,
                            )
                    eT = {}
                    for h in (he, ho):
                        eT[h] = epool.tile([128, 2, NPAIR], BF16, tag="eT", name=f"eT{h}")
                        nc.scalar.activation(
                            eT[h][:, 0, :],
                            psA[h][:],
                            mybir.ActivationFunctionType.Exp,
                            scale=0.125,
                        )
                        nc.scalar.activation(
                            eT[h][0:68, 1, :],
                            psB[h][0:68, :],
                            mybir.ActivationFunctionType.Exp,
                            scale=0.125,
                        )

                    # r = colsum(eT), replicated onto 64 rows per head via
                    # ones64 lhsT; divisor = exp(-ln(r)) on ACT
                    ps_r = ps_s.tile([128, NPAIR], F32, tag="sT", name="ps_r")
                    for h, base in ((he, 0), (ho, 64)):
                        for kch, kn in ((0, 128), (1, 68)):
                            nc.tensor.matmul(
                                ps_r[ds(base, 64), :],
                                ones64[0:kn, :],
                                eT[h][0:kn, kch, :],
                                start=(kch == 0),
                                stop=(kch == 1),
                            )
                    div_sb = dpool.tile([128, NPAIR], F32, tag="div")
                    nc.vector.reciprocal_approx_fast(div_sb[:], ps_r[:])

                    # u (e-weighted v, transposed out) and av (static bias term)
                    ps_u = ps_uav.tile([128, NPAIR], F32, tag="uav")
                    ps_av = ps_uav.tile([128, NPAIR], F32, tag="uav")
                    for h, base in ((he, 0), (ho, 64)):
                        for b01 in range(2):
                            bcol = b01 * N
                            for kch, kn in ((0, 128), (1, 68)):
                                vv = v_g[0:kn, b01, kch, ds(h * 64, 64)]
                                nc.tensor.matmul(
                                    ps_u[ds(base, 64), ds(bcol, N)],
                                    vv,
                                    eT[h][0:kn, kch, ds(bcol, N)],
                                    start=(kch == 0),
                                    stop=(kch == 1),
                                )
                                nc.tensor.matmul(
                                    ps_av[ds(base, 64), ds(bcol, N)],
                                    vv,
                                    aT_sb[0:kn, h, kch, :],
                                    start=(kch == 0),
                                    stop=(kch == 1),
                                )
                    nc.vector.tensor_mul(ocat[:, j, :], ps_u[:], div_sb[:])
                    nc.vector.tensor_add(ocat[:, j, :], ocat[:, j, :], ps_av[:])

                # --- output projection [tok, 768] ---
                for m_idx in range(4):
                    b01, half = divmod(m_idx, 2)
                    toff = b01 * N + half * 128
                    tm = 128 if half == 0 else 68
                    pp1 = ps_mm.tile([128, 512], F32, tag="mm")
                    pp2 = ps_mm.tile([128, 512], F32, tag="mm")
                    for j in range(KCH):
                        lhsT = ocat[:, j, ds(toff, tm)]
                        nc.tensor.matmul(
                            pp1[0:tm, 0:512],
                            lhsT,
                            Wp[:, j, 0:512],
                            start=(j == 0),
                            stop=(j == KCH - 1),
                        )
                        nc.tensor.matmul(
                            pp2[0:tm, 0:256],
                            lhsT,
                            Wp[:, j, 512:768],
                            start=(j == 0),
                            stop=(j == KCH - 1),
                        )
                    osb = opool.tile([128, C], F32)
                    nc.vector.tensor_add(
                        osb[0:tm, 0:512], pp1[0:tm, 0:512], bias_p[0:tm, 0:512]
                    )
                    nc.vector.tensor_add(
                        osb[0:tm, 512:768], pp2[0:tm, 0:256], bias_p[0:tm, 512:768]
                    )
                    nc.sync.dma_start(
                        out_d[ds(gcol + toff, tm), :], osb[0:tm, :]
                    )

    nc.compile()
    return nc


def _prep_in_maps(x, Wqkv, bqkv, Wproj, bproj, static_a):
    x = np.asarray(x, dtype=np.float32)
    Wqkv = np.asarray(Wqkv, dtype=np.float32)
    bqkv = np.asarray(bqkv, dtype=np.float32)
    Wproj = np.asarray(Wproj, dtype=np.float32)
    bproj = np.asarray(bproj, dtype=np.float32)
    static_a = np.asarray(static_a, dtype=np.float32)

    wqkvT = np.ascontiguousarray(Wqkv.T).astype(ml_dtypes.bfloat16)
    wprojT = np.ascontiguousarray(Wproj.T).astype(ml_dtypes.bfloat16)
    # aT packed for single-DMA load: aTp[p, h, ch, n] = static_a[0,h].T[ch*128+p, n]
    aTt = static_a[0].transpose(0, 2, 1)  # [H, m, n]
    aTp = np.zeros((128, H, 2, N), dtype=np.float32)
    aTp[:, :, 0, :] = aTt.transpose(1, 0, 2)[0:128]
    aTp[0:68, :, 1, :] = aTt.transpose(1, 0, 2)[128:N]
    aTp = aTp.astype(ml_dtypes.bfloat16)
    bqkv_qk = np.ascontiguousarray(bqkv[0:1536].reshape(12, 128).T)
    bias_v = np.broadcast_to(bqkv[1536:], (128, C)).copy()
    bias_p = np.broadcast_to(bproj, (128, C)).copy()

    in_maps = []
    for i in range(N_CORES):
        xc = x[i * B_PER_CORE : (i + 1) * B_PER_CORE]  # [8, 196, 768]
        xT = np.ascontiguousarray(xc.transpose(2, 0, 1).reshape(C, TOK)).astype(
            ml_dtypes.bfloat16
        )
        in_maps.append(
            {
                "xT": xT,
                "wqkvT": wqkvT,
                "bqkv_qk": bqkv_qk,
                "wprojT": wprojT,
                "bias_v": bias_v,
                "bias_p": bias_p,
                "aTp": aTp,
            }
        )
    return in_maps


def kernel(x, Wqkv, bqkv, Wproj, bproj, static_a, _trace=False, _trace_kwargs=None):
    if "nc" not in _BUILD_CACHE:
        _BUILD_CACHE["nc"] = build_nc()
    nc = _BUILD_CACHE["nc"]
    in_maps = _prep_in_maps(x, Wqkv, bqkv, Wproj, bproj, static_a)
    res = run_bass_kernel_spmd(
        nc,
        in_maps,
        core_ids=list(range(N_CORES)),
        trace=_trace,
        **(_trace_kwargs or {}),
    )
    outs = [res.results[i]["out"].reshape(B_PER_CORE, N, C) for i in range(N_CORES)]
    full = np.concatenate(outs, axis=0).astype(np.float32)
    if _trace:
        kernel.last_results = res
    return full
